# revision 10
# baseline (speedup 1.0000x reference)
"""Multi-head cross-attention Trainium2 kernel (8 NeuronCores).

Problem shapes (hardcoded): query (4,512,256); key_value (4,256,64,64);
Wq/Wk/Wv/Wo (256,256); biases (256,). NUM_HEADS=8, HEAD_DIM=32.

Sharding: 8 cores = 4 batches x 2 head-groups (4 heads / 128 dims each).
Each core computes its head-group's attention for one batch plus the
partial output projection over its 128 contraction dims; the host adds
the two partials per batch plus (bv @ Wo.T + bo), which is exactly the
missing bias terms (softmax is invariant to bk; bv passes through the
attention weights unchanged).

Per-core dataflow (S^T layout: kv position j on partitions, s on free):
  kv block [256, 512] --DMA-->
  K^T[dk,j]  = WkT.T @ kv      (PE, fp32r)
  V[j,dv]    = kv.T @ WvT      (PE, fp32r)
  S^T[j,s]   = KT_h.T @ QT_h   (PE, K=32 row-tiled 4 heads concurrent)
  P^T        = exp(scale*S^T)  (ACT, PSUM->SBUF, one pass -- the bottleneck)
  out^T[dv,s]+= V_h.T @ P^T    (PE, M=32 col-tiled, PSUM-accumulated)
  sum[h,s]  += ones.T @ P^T    (PE, col-tiled, rows broadcast within head)
  attn^T     = out^T * exp(-ln(sum))        (ACT+DVE)
  out[s,do]  = attn^T.T @ WoT  (PE) --DMA--> DRAM
Softmax max-subtraction is skipped: scores are ~N(0,1) after the 1/sqrt(32)
scale, so exp() is well within fp32 range; results match jax.nn.softmax to
fp rounding.
"""

import numpy as np

B, S, D = 4, 512, 256
HW = 4096
NHEADS_CORE = 4  # heads per core
HD = 32  # head dim
DC = 128  # head-group width in D
N_CORES = 8
SCALE = float(HD) ** -0.5

_PROG_CACHE = {}


def _build_program():
    from contextlib import ExitStack

    import concourse.bass as bass  # noqa: F401
    import concourse.tile as tile
    from concourse import bacc, masks, mybir

    f32 = mybir.dt.float32
    f32r = mybir.dt.float32r
    bf16 = mybir.dt.bfloat16  # noqa: F841
    fp16 = mybir.dt.float16
    AF = mybir.ActivationFunctionType

    def r(ap):
        return ap.bitcast(f32r)

    nc = bacc.Bacc("TRN2", target_bir_lowering=False, debug=False)

    q_d = nc.dram_tensor("q", [S, D], f32, kind="ExternalInput").ap()
    kv_d = nc.dram_tensor("kv", [D, HW], f32r, kind="ExternalInput").ap()
    wq_d = nc.dram_tensor("wq", [DC, D], f32, kind="ExternalInput").ap()
    wk_d = nc.dram_tensor("wk", [DC, D], f32, kind="ExternalInput").ap()
    wv_d = nc.dram_tensor("wv", [DC, D], f32, kind="ExternalInput").ap()
    wo_d = nc.dram_tensor("wo", [D, DC], f32, kind="ExternalInput").ap()
    bq_d = nc.dram_tensor("bq", [DC], f32, kind="ExternalInput").ap()
    out_d = nc.dram_tensor("out", [S, D], f32, kind="ExternalOutput").ap()

    with tile.TileContext(nc) as tc, ExitStack() as ctx:
        const_pool = ctx.enter_context(tc.tile_pool(name="const", bufs=1))
        wpool = ctx.enter_context(tc.tile_pool(name="wts", bufs=1))
        qpool = ctx.enter_context(tc.tile_pool(name="qstage", bufs=1))
        kvpool = ctx.enter_context(tc.tile_pool(name="kv", bufs=4))
        khpool = ctx.enter_context(tc.tile_pool(name="kh", bufs=4))
        ktpool = ctx.enter_context(tc.tile_pool(name="kt", bufs=2))
        vpool = ctx.enter_context(tc.tile_pool(name="v", bufs=2))
        ptpool = ctx.enter_context(tc.tile_pool(name="pt", bufs=4))
        mpool = ctx.enter_context(tc.tile_pool(name="misc", bufs=1))
        # PSUM budget: 3x[128,1024] work slots (6 banks) + attn acc (1) + sums (1)
        ps_work = ctx.enter_context(tc.tile_pool(name="psw", bufs=2, space="PSUM"))
        ps_kv = ctx.enter_context(tc.tile_pool(name="pskv", bufs=2, space="PSUM"))
        ps_att = ctx.enter_context(tc.tile_pool(name="psa", bufs=1, space="PSUM"))

        ident = const_pool.tile([128, 128], f32)
        masks.make_identity(nc, ident[:])

        # ---- weights: load + transpose (chunks along contraction dim d) ----
        def load_transposed(name, src_ap, dt_out):
            # src_ap: [128 rows, 256 cols] -> dst [128, 256] where
            # dst[:, 128c:128c+128] = src[:, 128c:128c+128].T
            raw = wpool.tile([128, 256], f32, tag=f"{name}raw")
            nc.sync.dma_start(raw[:], src_ap)
            dst = wpool.tile([128, 256], dt_out, tag=f"{name}T")
            for c in range(2):
                tp = ps_work.tile([128, 128], f32, tag="w")
                nc.tensor.transpose(tp[:], raw[:, 128 * c : 128 * (c + 1)], ident[:])
                dslice = dst[:, 128 * c : 128 * (c + 1)]
                if dt_out == f32:
                    dslice = r(dslice)
                nc.vector.tensor_copy(dslice, tp[:])
            return dst

        wqT = load_transposed("wq", wq_d[:, :], f32)
        wkT = load_transposed("wk", wk_d[:, :], fp16)
        wvT = load_transposed("wv", wv_d[:, :], fp16)
        # Wo[:, slice] is [256, 128]; load row-chunks side by side then transpose
        wo_raw = wpool.tile([128, 256], f32, tag="woraw")
        nc.sync.dma_start(wo_raw[:, 0:128], wo_d[0:128, :])
        nc.sync.dma_start(wo_raw[:, 128:256], wo_d[128:256, :])
        woT = wpool.tile([128, 256], fp16, tag="woT")  # [dc, do]
        for u in range(2):
            tp = ps_work.tile([128, 128], f32, tag="w")
            nc.tensor.transpose(tp[:], wo_raw[:, 128 * u : 128 * (u + 1)], ident[:])
            nc.vector.tensor_copy(woT[:, 128 * u : 128 * (u + 1)], tp[:])

        bq_sb = wpool.tile([128, 1], f32, tag="bq")
        nc.sync.dma_start(bq_sb[:], bq_d.unsqueeze(1))

        # ---- query: load, transpose to [d, s], project to Q^T [dq, s] ----
        q_sb = qpool.tile([128, 1024], f32, tag="qraw")  # 4 s-chunks of [128,256]
        for sc in range(4):
            nc.sync.dma_start(
                q_sb[:, 256 * sc : 256 * (sc + 1)], q_d[128 * sc : 128 * (sc + 1), :]
            )
        qT = qpool.tile([128, 1024], f32, tag="qT")  # 2 d-chunks of [128, 512]
        for c in range(2):
            for sc in range(4):
                tp = ps_work.tile([128, 128], f32, tag="w")
                nc.tensor.transpose(
                    tp[:], q_sb[:, 256 * sc + 128 * c : 256 * sc + 128 * (c + 1)],
                    ident[:],
                )
                nc.vector.tensor_copy(
                    r(qT[:, 512 * c + 128 * sc : 512 * c + 128 * (sc + 1)]), tp[:]
                )
        qt_ps = ps_work.tile([128, 512], f32, tag="w")
        for c in range(2):
            nc.tensor.matmul(
                qt_ps[:],
                r(wqT[:, 128 * c : 128 * (c + 1)]),
                r(qT[:, 512 * c : 512 * (c + 1)]),
                start=(c == 0),
                stop=(c == 1),
            )
        QT = qpool.tile([128, 512], fp16, tag="QT")
        nc.vector.tensor_scalar_add(QT[:], qt_ps[:], bq_sb[:])

        # ---- main streaming loop over kv position blocks ----
        # att accumulator [128, 1024]: head h -> [64*(h%2) : +64, 512*(h//2) : +512]
        # rows 0-31 of each 64-block = attn out^T, rows 32-63 = sumexp (bcast)
        att_ps = ps_att.tile([128, 1024], f32)

        for jc in range(8):  # 512-wide kv blocks
            kv0 = kvpool.tile([128, 512], f32, tag="kv")
            kv1 = kvpool.tile([128, 512], f32, tag="kv")
            nc.sync.dma_start(r(kv0[:]), kv_d[0:128, 512 * jc : 512 * (jc + 1)])
            nc.sync.dma_start(r(kv1[:]), kv_d[128:256, 512 * jc : 512 * (jc + 1)])
            # fp16 casts feed all PE consumers of kv
            kh0 = khpool.tile([128, 512], fp16, tag="kh")
            kh1 = khpool.tile([128, 512], fp16, tag="kh")
            nc.vector.tensor_copy(kh0[:], kv0[:])
            nc.vector.tensor_copy(kh1[:], kv1[:])
            khc = (kh0, kh1)

            # K^T block [dk=128, j=512] (fp16 inputs, fp32 psum)
            kt_ps = ps_kv.tile([128, 512], f32, tag="kvp")
            for c in range(2):
                nc.tensor.matmul(
                    kt_ps[:],
                    wkT[:, 128 * c : 128 * (c + 1)],
                    khc[c][:],
                    start=(c == 0),
                    stop=(c == 1),
                )
            kt_sb = ktpool.tile([128, 512], fp16, tag="kt")
            nc.vector.tensor_copy(kt_sb[:], kt_ps[:])

            # V block -> v_sb [128, 1024] interleaved per jsub/head:
            #   cols [256*jsub + 64*h : +32] = V_h, [.. +32 : +64] = ones
            v_ps = ps_kv.tile([128, 512], f32, tag="kvp")
            for js in range(4):
                for c in range(2):
                    nc.tensor.matmul(
                        v_ps[:, 128 * js : 128 * (js + 1)],
                        khc[c][:, 128 * js : 128 * (js + 1)],
                        wvT[:, 128 * c : 128 * (c + 1)],
                        start=(c == 0),
                        stop=(c == 1),
                    )
            v_sb = vpool.tile([128, 1024], fp16, tag="v")
            # ones columns (16 groups of 32 at stride 64, offset 32)
            nc.vector.memset(
                v_sb[:].rearrange("p (g two x) -> p g two x", two=2, x=32)[:, :, 1, :],
                1.0,
            )
            for js in range(4):
                nc.vector.tensor_copy(
                    v_sb[:, 256 * js : 256 * (js + 1)].rearrange(
                        "p (h two x) -> p h two x", two=2, x=32
                    )[:, :, 0, :],
                    v_ps[:, 128 * js : 128 * (js + 1)].rearrange(
                        "p (h x) -> p h x", x=32
                    ),
                )

            for js in range(4):  # 128-wide j waves
                first = jc == 0 and js == 0
                last = jc == 7 and js == 3
                pts = []
                for hp in range(2):  # head pairs -> [128, 1024] scores tiles
                    sc_ps = ps_work.tile([128, 1024], f32, tag="w")
                    for hh in range(2):
                        h = 2 * hp + hh
                        nc.tensor.matmul(
                            sc_ps[:, 512 * hh : 512 * (hh + 1)],
                            kt_sb[32 * h : 32 * (h + 1), 128 * js : 128 * (js + 1)],
                            QT[32 * h : 32 * (h + 1), :],
                            start=True,
                            stop=True,
                            tile_position=(32 * h, 0),
                        )
                    pt = ptpool.tile([128, 1024], fp16, tag="pt")
                    nc.scalar.activation(pt[:], sc_ps[:], AF.Exp, scale=SCALE)
                    pts.append(pt)
                for hp in range(2):
                    pt = pts[hp]
                    for hh in range(2):
                        h = 2 * hp + hh
                        nc.tensor.matmul(
                            att_ps[
                                64 * (h % 2) : 64 * (h % 2) + 64,
                                512 * (h // 2) : 512 * (h // 2) + 512,
                            ],
                            v_sb[:, 256 * js + 64 * h : 256 * js + 64 * (h + 1)],
                            pt[:, 512 * hh : 512 * (hh + 1)],
                            start=first,
                            stop=last,
                            tile_position=(0, 64 * (h % 2)),
                            # per-head groups touch disjoint partition ranges
                            # of the bank; the group lint is partition-unaware
                            skip_group_check=True,
                        )

        # ---- tail: normalize and project ----
        # gather per-head sums rows (shifted copies) into compact [128, 512]
        rs_raw = mpool.tile([128, 512], f32, tag="rsraw")
        att_c = mpool.tile([128, 512], f32, tag="attc")
        for h in range(4):
            pb = 64 * (h % 2)
            cb = 512 * (h // 2)
            nc.vector.tensor_copy(
                rs_raw[32 * h : 32 * (h + 1), :],
                att_ps[pb + 32 : pb + 64, cb : cb + 512],
            )
            nc.vector.tensor_copy(
                att_c[32 * h : 32 * (h + 1), :],
                att_ps[pb : pb + 32, cb : cb + 512],
            )
        lns = mpool.tile([128, 512], f32, tag="lns")
        nc.scalar.activation(lns[:], rs_raw[:], AF.Ln)
        rsum = mpool.tile([128, 512], f32, tag="rsum")
        nc.scalar.activation(rsum[:], lns[:], AF.Exp, scale=-1.0)
        attn = mpool.tile([128, 512], fp16, tag="attn")
        nc.vector.tensor_mul(attn[:], att_c[:], rsum[:])
        o_sb = mpool.tile([128, 1024], f32, tag="osb")
        for sc in range(4):
            o_ps = ps_work.tile([128, 1024], f32, tag="w")
            nc.tensor.matmul(
                o_ps[:, 0:256],
                attn[:, 128 * sc : 128 * (sc + 1)],
                woT[:],
                start=True,
                stop=True,
            )
            o_slice = o_sb[:, 256 * sc : 256 * (sc + 1)]
            nc.vector.tensor_copy(o_slice, o_ps[:, 0:256])
            nc.sync.dma_start(out_d[128 * sc : 128 * (sc + 1), :], o_slice)

    nc.compile()
    return nc


def get_program():
    if "nc" not in _PROG_CACHE:
        _PROG_CACHE["nc"] = _build_program()
    return _PROG_CACHE["nc"]


def make_in_maps(query, key_value, Wq, bq, Wk, bk, Wv, bv, Wo, bo):
    query = np.ascontiguousarray(np.asarray(query, dtype=np.float32))
    key_value = np.ascontiguousarray(np.asarray(key_value, dtype=np.float32))
    Wq = np.asarray(Wq, dtype=np.float32)
    Wk = np.asarray(Wk, dtype=np.float32)
    Wv = np.asarray(Wv, dtype=np.float32)
    Wo = np.asarray(Wo, dtype=np.float32)
    bq = np.asarray(bq, dtype=np.float32)
    in_maps = []
    for c in range(N_CORES):
        b, g = c // 2, c % 2
        sl = slice(g * DC, (g + 1) * DC)
        in_maps.append(
            {
                "q": query[b],
                "kv": np.ascontiguousarray(key_value[b].reshape(D, HW)),
                "wq": np.ascontiguousarray(Wq[sl]),
                "wk": np.ascontiguousarray(Wk[sl]),
                "wv": np.ascontiguousarray(Wv[sl]),
                "wo": np.ascontiguousarray(Wo[:, sl]),
                "bq": np.ascontiguousarray(bq[sl]),
            }
        )
    return in_maps


def run_on_cores(in_maps, trace=False):
    from concourse import bass_utils

    nc = get_program()
    return bass_utils.run_bass_kernel_spmd(
        nc, in_maps, core_ids=list(range(N_CORES)), trace=trace
    )


def kernel(query, key_value, Wq, bq, Wk, bk, Wv, bv, Wo, bo):
    in_maps = make_in_maps(query, key_value, Wq, bq, Wk, bk, Wv, bv, Wo, bo)
    res = run_on_cores(in_maps)
    Wo_np = np.asarray(Wo, dtype=np.float32)
    bias = np.asarray(bv, dtype=np.float32) @ Wo_np.T + np.asarray(
        bo, dtype=np.float32
    )
    out = np.empty((B, S, D), dtype=np.float32)
    for b in range(B):
        out[b] = res.results[2 * b]["out"] + res.results[2 * b + 1]["out"] + bias
    return out
